# revision 4
# baseline (speedup 1.0000x reference)
"""Fused FP8-block-quantized MLP (silu(x@w1.T) * (x@w3.T)) @ w2.T on 8 trn2 cores.

Sharding: data-parallel over tokens. Each core gets T/8 = 512 tokens and the
full (dequantized, bf16) weights; there are no collectives. Host-side prep
dequantizes the block-quantized weights, casts to bf16, and lays tensors out
partition-major so every device DMA is one large contiguous transfer.

Device kernel per core (all matmuls bf16, fp32 PSUM accumulation):
  warmup:  a burst of dummy matmuls on a zeroed scratch tile keeps the PE
           busy from the end of the engine prologue, so the HAM clock gate
           reaches 8/8 (~3.4us of sustained activity) before real data
           lands and the first real matmuls run at full clock.
  phase A: for each 128-row block fb of F: g.T/u.T [128f, 512t] accumulated
           over 16 k-blocks of H; silu on ACT; DVE multiplies silu(g) with
           u straight out of PSUM -> fusedT kept in SBUF.
  phase B: out [512t, 2048h] = fusedT.T @ w2.T, streaming w2 in 512KB
           4-block tiles, accumulating over the 56 f-blocks in PSUM.

Startup DMAs are ordered so the first matmul needs only 160KB (x k-block 0
plus the first quarter of w1) instead of 768KB, with the rest of x/w1/w3
following in consumption order.
"""

import sys

import numpy as np

_REPO = "/opt/trn_rl_repo"
if _REPO not in sys.path:
    sys.path.insert(0, _REPO)

T, H, F = 4096, 2048, 7168
NCORES = 8
TC = T // NCORES      # 512 tokens per core
KB = H // 128         # 16 contraction blocks for matmul 1/3
FB = F // 128         # 56 f blocks
W2G = 4               # f-blocks per w2 DMA tile
W2J = FB // W2G       # 14 w2 tiles per output column group
HCOLS = H // 512      # 4 output column groups
TB = TC // 128        # 4 token blocks
NWARM = 9             # HAM warmup matmuls: ~3.8us span, ends as data arrives

_CACHE = {}


def _build_program():
    import concourse.mybir as mybir
    from concourse import bacc
    from concourse.tile import TileContext

    bf16 = mybir.dt.bfloat16
    f32 = mybir.dt.float32

    # Bacc (not bass.Bass): its finalize() runs generate_event_semaphores,
    # which splits multi-wait sync_info into EventSemaphore instructions -
    # TRN2 instructions physically carry at most one sem wait.
    nc = bacc.Bacc()
    xt_d = nc.declare_dram_parameter("xt", [128, KB, TC], bf16, isOutput=False)
    w13_d = nc.declare_dram_parameter(
        "w13p", [FB, 128, 2, H], bf16, isOutput=False
    )
    w2_d = nc.declare_dram_parameter(
        "w2p", [HCOLS, W2J, 128, W2G, 512], bf16, isOutput=False
    )
    out_d = nc.declare_dram_parameter("out", [TC, H], f32, isOutput=True)

    with TileContext(nc) as tc:
        with (
            tc.tile_pool(name="xpool", bufs=1) as xpool,
            tc.tile_pool(name="spool", bufs=1) as spool,
            tc.tile_pool(name="wpool", bufs=2) as wpool,
            tc.tile_pool(name="w2pool", bufs=4) as w2pool,
            tc.tile_pool(name="sgpool", bufs=3) as sgpool,
            tc.tile_pool(name="fpool", bufs=FB) as fpool,
            tc.tile_pool(name="opool", bufs=HCOLS * TB) as opool,
            tc.tile_pool(name="psg", bufs=2, space="PSUM") as psg,
            tc.tile_pool(name="psu", bufs=2, space="PSUM") as psu,
            tc.tile_pool(name="psb", bufs=4, space="PSUM") as psb,
        ):
            scratch = spool.tile([128, TC], bf16)
            xtile = xpool.tile([128, KB, TC], bf16)

            # PE warmup: depends only on the memset, so it runs as soon as
            # the engine prologue ends and un-throttles the HAM clock gate
            # while the startup DMAs are still in flight.
            nc.vector.memset(scratch, 0.0)
            wps = psg.tile([128, TC], f32, tag="gps", name="warmps")
            for _ in range(NWARM):
                nc.tensor.matmul(
                    wps,
                    scratch[:, 0:128],
                    scratch,
                    start=True,
                    stop=True,
                    skip_group_check=True,
                )

            fused = []
            for fb in range(FB):
                w13t = wpool.tile([128, 2, H], bf16, tag="w13t")
                if fb == 0:
                    # Startup pieces in consumption order: the first matmul
                    # needs just x[kb0] + w1[k<512]; later pieces stream in
                    # while the PE (already warm) chews the early k-blocks.
                    nc.sync.dma_start(
                        out=xtile[:, 0:1, :], in_=xt_d[:, 0:1, :]
                    )
                    nc.sync.dma_start(
                        out=w13t[:, 0, 0:512], in_=w13_d[0][:, 0, 0:512]
                    )
                    nc.sync.dma_start(
                        out=xtile[:, 1:4, :], in_=xt_d[:, 1:4, :]
                    )
                    nc.sync.dma_start(
                        out=w13t[:, 0, 512:1024], in_=w13_d[0][:, 0, 512:1024]
                    )
                    nc.sync.dma_start(
                        out=xtile[:, 4:8, :], in_=xt_d[:, 4:8, :]
                    )
                    nc.sync.dma_start(
                        out=w13t[:, 0, 1024:2048], in_=w13_d[0][:, 0, 1024:2048]
                    )
                    nc.sync.dma_start(
                        out=xtile[:, 8:16, :], in_=xt_d[:, 8:16, :]
                    )
                    nc.sync.dma_start(
                        out=w13t[:, 1, 0:1024], in_=w13_d[0][:, 1, 0:1024]
                    )
                    nc.sync.dma_start(
                        out=w13t[:, 1, 1024:2048], in_=w13_d[0][:, 1, 1024:2048]
                    )
                else:
                    nc.sync.dma_start(out=w13t, in_=w13_d[fb])

                gps = psg.tile([128, TC], f32, tag="gps")
                for kb in range(KB):
                    nc.tensor.matmul(
                        gps,
                        w13t[:, 0, kb * 128 : (kb + 1) * 128],
                        xtile[:, kb, :],
                        start=(kb == 0),
                        stop=(kb == KB - 1),
                    )
                ups = psu.tile([128, TC], f32, tag="ups")
                for kb in range(KB):
                    nc.tensor.matmul(
                        ups,
                        w13t[:, 1, kb * 128 : (kb + 1) * 128],
                        xtile[:, kb, :],
                        start=(kb == 0),
                        stop=(kb == KB - 1),
                    )

                sg = sgpool.tile([128, TC], f32, tag="sg")
                nc.scalar.activation(
                    sg, gps, mybir.ActivationFunctionType.Silu
                )
                fut = fpool.tile(
                    [128, TC], bf16, tag="fused", name=f"fused{fb}"
                )
                # DVE multiplies u straight out of PSUM; no ACT copy needed.
                nc.vector.tensor_tensor(
                    fut, sg, ups, mybir.AluOpType.mult
                )
                fused.append(fut)

            for hc in range(HCOLS):
                pss = []
                for tb in range(TB):
                    ps = psb.tile(
                        [128, 512], f32, tag="pss", name=f"pss{hc}_{tb}"
                    )
                    pss.append(ps)
                for j in range(W2J):
                    w2t = w2pool.tile([128, W2G, 512], bf16, tag="w2t")
                    nc.sync.dma_start(out=w2t, in_=w2_d[hc, j])
                    for i in range(W2G):
                        fb = W2G * j + i
                        for tb in range(TB):
                            nc.tensor.matmul(
                                pss[tb],
                                fused[fb][:, tb * 128 : (tb + 1) * 128],
                                w2t[:, i, :],
                                start=(fb == 0),
                                stop=(fb == FB - 1),
                            )
                # Evacuations split DVE/ACT so they drain in parallel;
                # stores split across both HWDGE rings to shorten the tail.
                for tb in range(TB):
                    ot = opool.tile(
                        [128, 512], f32, tag="ot", name=f"ot{hc}_{tb}"
                    )
                    if tb % 2 == 0:
                        nc.vector.tensor_copy(ot, pss[tb])
                    else:
                        nc.scalar.activation(
                            ot, pss[tb], mybir.ActivationFunctionType.Copy
                        )
                    eng = nc.sync if tb % 2 == 0 else nc.scalar
                    eng.dma_start(
                        out=out_d[
                            tb * 128 : (tb + 1) * 128,
                            hc * 512 : (hc + 1) * 512,
                        ],
                        in_=ot,
                    )
    nc.finalize()
    return nc


def _dequant(wq, s):
    wq = np.asarray(wq, dtype=np.float32)
    s = np.asarray(s, dtype=np.float32)
    n, k = wq.shape
    nb, kb = s.shape
    w = wq.reshape(nb, n // nb, kb, k // kb) * s[:, None, :, None]
    return w.reshape(n, k)


def _prep_inputs(hidden_states, w1_q, w1_s, w3_q, w3_s, w2_q, w2_s):
    import ml_dtypes

    bf = ml_dtypes.bfloat16

    w1 = _dequant(w1_q, w1_s).astype(bf)  # [F, H]
    w3 = _dequant(w3_q, w3_s).astype(bf)  # [F, H]
    w2 = _dequant(w2_q, w2_s).astype(bf)  # [H, F]

    # w1p[fb, p, kb*128+c] = w1[fb*128+c, kb*128+p]  (and same for w3);
    # interleaved per partition: w13p[fb, p, 0] = w1 row, [fb, p, 1] = w3.
    w1p = w1.reshape(FB, 128, KB, 128).transpose(0, 3, 2, 1).reshape(FB, 128, H)
    w3p = w3.reshape(FB, 128, KB, 128).transpose(0, 3, 2, 1).reshape(FB, 128, H)
    w13p = np.ascontiguousarray(np.stack([w1p, w3p], axis=2))  # [FB,128,2,H]

    # w2p[hc, j, p, i, c] = w2[hc*512+c, (W2G*j+i)*128+p]
    w2p = np.ascontiguousarray(
        np.asarray(w2).reshape(HCOLS, 512, W2J, W2G, 128).transpose(0, 2, 4, 3, 1)
    )

    x = np.asarray(hidden_states, dtype=np.float32).astype(bf)
    xts = []
    for c in range(NCORES):
        xc = x[c * TC : (c + 1) * TC, :]
        # xt[p, kb, t] = xc[t, kb*128+p] - partition-major, so the whole
        # 2MB x-transpose lands in one DMA with 16KB/partition contiguous.
        xts.append(
            np.ascontiguousarray(xc.reshape(TC, KB, 128).transpose(2, 1, 0))
        )

    return [
        {"xt": xts[c], "w13p": w13p, "w2p": w2p}
        for c in range(NCORES)
    ]


def _run(in_maps, **kwargs):
    from concourse.bass_utils import run_bass_kernel_spmd

    if "nc" not in _CACHE:
        _CACHE["nc"] = _build_program()
    res = run_bass_kernel_spmd(
        _CACHE["nc"], in_maps, list(range(NCORES)), **kwargs
    )
    out = np.concatenate(
        [res.results[c]["out"] for c in range(NCORES)], axis=0
    )
    return np.asarray(out, dtype=np.float32), res


def kernel(hidden_states, w1_q, w1_s, w3_q, w3_s, w2_q, w2_s):
    in_maps = _prep_inputs(
        hidden_states, w1_q, w1_s, w3_q, w3_s, w2_q, w2_s
    )
    out, _ = _run(in_maps)
    return out


# revision 6
# speedup vs baseline: 1.0138x; 1.0138x over previous
"""Fused FP8-block-quantized MLP (silu(x@w1.T) * (x@w3.T)) @ w2.T on 8 trn2 cores.

Sharding: data-parallel over tokens. Each core gets T/8 = 512 tokens and the
full (dequantized, bf16) weights; there are no collectives. Host-side prep
dequantizes the block-quantized weights, casts to bf16, and lays tensors out
partition-major so every device DMA is one large contiguous transfer.

Device kernel per core (all matmuls bf16, fp32 PSUM accumulation):
  warmup:  a burst of dummy matmuls on a zeroed scratch tile keeps the PE
           busy from the end of the engine prologue, so the HAM clock gate
           reaches 8/8 (~3.4us of sustained activity) before real data
           lands and the first real matmuls run at full clock.
  phase A: for each 128-row block fb of F: g.T/u.T [128f, 512t] accumulated
           over 16 k-blocks of H; silu on ACT; DVE multiplies silu(g) with
           u straight out of PSUM -> fusedT kept in SBUF.
  phase B: out [512t, 2048h] = fusedT.T @ w2.T, streaming w2 in 512KB
           4-block tiles, accumulating over the 56 f-blocks in PSUM.

Startup DMAs are ordered so the first matmul needs only 160KB (x k-block 0
plus the first quarter of w1) instead of 768KB, with the rest of x/w1/w3
following in consumption order.
"""

import sys

import numpy as np

_REPO = "/opt/trn_rl_repo"
if _REPO not in sys.path:
    sys.path.insert(0, _REPO)

T, H, F = 4096, 2048, 7168
NCORES = 8
TC = T // NCORES      # 512 tokens per core
KB = H // 128         # 16 contraction blocks for matmul 1/3
FB = F // 128         # 56 f blocks
W2G = 4               # f-blocks per w2 DMA tile
W2J = FB // W2G       # 14 w2 tiles per output column group
HCOLS = H // 512      # 4 output column groups
TB = TC // 128        # 4 token blocks
NWARM = 14            # HAM warmup matmuls: span ends ~when startup data lands

_CACHE = {}


def _build_program():
    import concourse.mybir as mybir
    from concourse import bacc
    from concourse.tile import TileContext

    bf16 = mybir.dt.bfloat16
    f32 = mybir.dt.float32

    # Bacc (not bass.Bass): its finalize() runs generate_event_semaphores,
    # which splits multi-wait sync_info into EventSemaphore instructions -
    # TRN2 instructions physically carry at most one sem wait.
    nc = bacc.Bacc()
    xt_d = nc.declare_dram_parameter("xt", [128, KB, TC], bf16, isOutput=False)
    w13_d = nc.declare_dram_parameter(
        "w13p", [FB, 128, 2, H], bf16, isOutput=False
    )
    w2_d = nc.declare_dram_parameter(
        "w2p", [HCOLS, W2J, 128, W2G, 512], bf16, isOutput=False
    )
    out_d = nc.declare_dram_parameter("out", [TC, H], f32, isOutput=True)

    with TileContext(nc) as tc:
        with (
            tc.tile_pool(name="xpool", bufs=1) as xpool,
            tc.tile_pool(name="spool", bufs=1) as spool,
            tc.tile_pool(name="wpool", bufs=2) as wpool,
            tc.tile_pool(name="w2pool", bufs=4) as w2pool,
            tc.tile_pool(name="sgpool", bufs=3) as sgpool,
            tc.tile_pool(name="fpool", bufs=FB) as fpool,
            tc.tile_pool(name="opool", bufs=HCOLS * TB) as opool,
            tc.tile_pool(name="psg", bufs=2, space="PSUM") as psg,
            tc.tile_pool(name="psu", bufs=2, space="PSUM") as psu,
            tc.tile_pool(name="psb", bufs=4, space="PSUM") as psb,
        ):
            scratch = spool.tile([128, TC], bf16)
            xtile = xpool.tile([128, KB, TC], bf16)

            # PE warmup: depends only on the memset, so it runs as soon as
            # the engine prologue ends and un-throttles the HAM clock gate
            # while the startup DMAs are still in flight.
            nc.vector.memset(scratch, 0.0)
            wps = psg.tile([128, TC], f32, tag="gps", name="warmps")
            for _ in range(NWARM):
                nc.tensor.matmul(
                    wps,
                    scratch[:, 0:128],
                    scratch,
                    start=True,
                    stop=True,
                    skip_group_check=True,
                )

            fused = []
            for fb in range(FB):
                w13t = wpool.tile([128, 2, H], bf16, tag="w13t")
                if fb == 0:
                    # Startup quarters: x on the scalar HWDGE ring, w13[0]
                    # on the sync ring, so both streams load in parallel
                    # and quarter-pair q is ready before the (warm) PE
                    # finishes chewing quarter q-1.
                    kq, hq = KB // 4, H // 4
                    for q in range(4):
                        nc.scalar.dma_start(
                            out=xtile[:, q * kq : (q + 1) * kq, :],
                            in_=xt_d[:, q * kq : (q + 1) * kq, :],
                        )
                        nc.sync.dma_start(
                            out=w13t[:, :, q * hq : (q + 1) * hq],
                            in_=w13_d[0][:, :, q * hq : (q + 1) * hq],
                        )
                else:
                    nc.sync.dma_start(out=w13t, in_=w13_d[fb])

                # g/u interleaved per k-block: consumption tracks the
                # arrival order of the startup quarters.
                gps = psg.tile([128, TC], f32, tag="gps")
                ups = psu.tile([128, TC], f32, tag="ups")
                for kb in range(KB):
                    nc.tensor.matmul(
                        gps,
                        w13t[:, 0, kb * 128 : (kb + 1) * 128],
                        xtile[:, kb, :],
                        start=(kb == 0),
                        stop=(kb == KB - 1),
                        skip_group_check=True,
                    )
                    nc.tensor.matmul(
                        ups,
                        w13t[:, 1, kb * 128 : (kb + 1) * 128],
                        xtile[:, kb, :],
                        start=(kb == 0),
                        stop=(kb == KB - 1),
                        skip_group_check=True,
                    )

                sg = sgpool.tile([128, TC], f32, tag="sg")
                nc.scalar.activation(
                    sg, gps, mybir.ActivationFunctionType.Silu
                )
                fut = fpool.tile(
                    [128, TC], bf16, tag="fused", name=f"fused{fb}"
                )
                # DVE multiplies u straight out of PSUM; no ACT copy needed.
                nc.vector.tensor_tensor(
                    fut, sg, ups, mybir.AluOpType.mult
                )
                fused.append(fut)

            for hc in range(HCOLS):
                pss = []
                for tb in range(TB):
                    ps = psb.tile(
                        [128, 512], f32, tag="pss", name=f"pss{hc}_{tb}"
                    )
                    pss.append(ps)
                for j in range(W2J):
                    w2t = w2pool.tile([128, W2G, 512], bf16, tag="w2t")
                    nc.sync.dma_start(out=w2t, in_=w2_d[hc, j])
                    for i in range(W2G):
                        fb = W2G * j + i
                        for tb in range(TB):
                            nc.tensor.matmul(
                                pss[tb],
                                fused[fb][:, tb * 128 : (tb + 1) * 128],
                                w2t[:, i, :],
                                start=(fb == 0),
                                stop=(fb == FB - 1),
                            )
                # Evacuations split DVE/ACT so they drain in parallel;
                # stores split across both HWDGE rings to shorten the tail.
                for tb in range(TB):
                    ot = opool.tile(
                        [128, 512], f32, tag="ot", name=f"ot{hc}_{tb}"
                    )
                    if tb % 2 == 0:
                        nc.vector.tensor_copy(ot, pss[tb])
                    else:
                        nc.scalar.activation(
                            ot, pss[tb], mybir.ActivationFunctionType.Copy
                        )
                    eng = nc.sync if tb % 2 == 0 else nc.scalar
                    eng.dma_start(
                        out=out_d[
                            tb * 128 : (tb + 1) * 128,
                            hc * 512 : (hc + 1) * 512,
                        ],
                        in_=ot,
                    )
    nc.finalize()
    return nc


def _dequant(wq, s):
    wq = np.asarray(wq, dtype=np.float32)
    s = np.asarray(s, dtype=np.float32)
    n, k = wq.shape
    nb, kb = s.shape
    w = wq.reshape(nb, n // nb, kb, k // kb) * s[:, None, :, None]
    return w.reshape(n, k)


def _prep_inputs(hidden_states, w1_q, w1_s, w3_q, w3_s, w2_q, w2_s):
    import ml_dtypes

    bf = ml_dtypes.bfloat16

    w1 = _dequant(w1_q, w1_s).astype(bf)  # [F, H]
    w3 = _dequant(w3_q, w3_s).astype(bf)  # [F, H]
    w2 = _dequant(w2_q, w2_s).astype(bf)  # [H, F]

    # w1p[fb, p, kb*128+c] = w1[fb*128+c, kb*128+p]  (and same for w3);
    # interleaved per partition: w13p[fb, p, 0] = w1 row, [fb, p, 1] = w3.
    w1p = w1.reshape(FB, 128, KB, 128).transpose(0, 3, 2, 1).reshape(FB, 128, H)
    w3p = w3.reshape(FB, 128, KB, 128).transpose(0, 3, 2, 1).reshape(FB, 128, H)
    w13p = np.ascontiguousarray(np.stack([w1p, w3p], axis=2))  # [FB,128,2,H]

    # w2p[hc, j, p, i, c] = w2[hc*512+c, (W2G*j+i)*128+p]
    w2p = np.ascontiguousarray(
        np.asarray(w2).reshape(HCOLS, 512, W2J, W2G, 128).transpose(0, 2, 4, 3, 1)
    )

    x = np.asarray(hidden_states, dtype=np.float32).astype(bf)
    xts = []
    for c in range(NCORES):
        xc = x[c * TC : (c + 1) * TC, :]
        # xt[p, kb, t] = xc[t, kb*128+p] - partition-major, so the whole
        # 2MB x-transpose lands in one DMA with 16KB/partition contiguous.
        xts.append(
            np.ascontiguousarray(xc.reshape(TC, KB, 128).transpose(2, 1, 0))
        )

    return [
        {"xt": xts[c], "w13p": w13p, "w2p": w2p}
        for c in range(NCORES)
    ]


def _run(in_maps, **kwargs):
    from concourse.bass_utils import run_bass_kernel_spmd

    if "nc" not in _CACHE:
        _CACHE["nc"] = _build_program()
    res = run_bass_kernel_spmd(
        _CACHE["nc"], in_maps, list(range(NCORES)), **kwargs
    )
    out = np.concatenate(
        [res.results[c]["out"] for c in range(NCORES)], axis=0
    )
    return np.asarray(out, dtype=np.float32), res


def kernel(hidden_states, w1_q, w1_s, w3_q, w3_s, w2_q, w2_s):
    in_maps = _prep_inputs(
        hidden_states, w1_q, w1_s, w3_q, w3_s, w2_q, w2_s
    )
    out, _ = _run(in_maps)
    return out


# revision 8
# speedup vs baseline: 1.0163x; 1.0025x over previous
"""Fused FP8-block-quantized MLP (silu(x@w1.T) * (x@w3.T)) @ w2.T on 8 trn2 cores.

Sharding: data-parallel over tokens. Each core gets T/8 = 512 tokens and the
full (dequantized, bf16) weights; there are no collectives. Host-side prep
dequantizes the block-quantized weights, casts to bf16, and lays tensors out
partition-major so every device DMA is one large contiguous transfer.

Device kernel per core (all matmuls bf16, fp32 PSUM accumulation):
  warmup:  a burst of dummy matmuls on a zeroed scratch tile keeps the PE
           busy from the end of the engine prologue, so the HAM clock gate
           reaches 8/8 (~3.4us of sustained activity) before real data
           lands and the first real matmuls run at full clock.
  phase A: for each 128-row block fb of F: g.T/u.T [128f, 512t] accumulated
           over 16 k-blocks of H; silu on ACT; DVE multiplies silu(g) with
           u straight out of PSUM -> fusedT kept in SBUF.
  phase B: out [512t, 2048h] = fusedT.T @ w2.T, streaming w2 in 512KB
           4-block tiles, accumulating over the 56 f-blocks in PSUM.

Startup DMAs are ordered so the first matmul needs only 160KB (x k-block 0
plus the first quarter of w1) instead of 768KB, with the rest of x/w1/w3
following in consumption order.
"""

import sys

import numpy as np

_REPO = "/opt/trn_rl_repo"
if _REPO not in sys.path:
    sys.path.insert(0, _REPO)

T, H, F = 4096, 2048, 7168
NCORES = 8
TC = T // NCORES      # 512 tokens per core
KB = H // 128         # 16 contraction blocks for matmul 1/3
FB = F // 128         # 56 f blocks
W2G = 4               # f-blocks per w2 DMA tile
W2J = FB // W2G       # 14 w2 tiles per output column group
HCOLS = H // 512      # 4 output column groups
TB = TC // 128        # 4 token blocks
NWARM = 8             # HAM warmup matmuls: ~3.4us busy, un-throttles the clock

_CACHE = {}


def _build_program():
    import concourse.mybir as mybir
    from concourse import bacc
    from concourse.tile import TileContext

    bf16 = mybir.dt.bfloat16
    f32 = mybir.dt.float32

    # Bacc (not bass.Bass): its finalize() runs generate_event_semaphores,
    # which splits multi-wait sync_info into EventSemaphore instructions -
    # TRN2 instructions physically carry at most one sem wait.
    nc = bacc.Bacc()
    xt_d = nc.declare_dram_parameter("xt", [128, KB, TC], bf16, isOutput=False)
    w13_d = nc.declare_dram_parameter(
        "w13p", [FB, 128, 2, H], bf16, isOutput=False
    )
    w2_d = nc.declare_dram_parameter(
        "w2p", [HCOLS, W2J, 128, W2G, 512], bf16, isOutput=False
    )
    out_d = nc.declare_dram_parameter("out", [TC, H], f32, isOutput=True)

    with TileContext(nc) as tc:
        with (
            tc.tile_pool(name="xpool", bufs=1) as xpool,
            tc.tile_pool(name="spool", bufs=1) as spool,
            tc.tile_pool(name="wpool", bufs=2) as wpool,
            tc.tile_pool(name="w2pool", bufs=4) as w2pool,
            tc.tile_pool(name="sgpool", bufs=3) as sgpool,
            tc.tile_pool(name="fpool", bufs=FB) as fpool,
            tc.tile_pool(name="opool", bufs=HCOLS * TB) as opool,
            tc.tile_pool(name="psg", bufs=2, space="PSUM") as psg,
            tc.tile_pool(name="psu", bufs=2, space="PSUM") as psu,
            tc.tile_pool(name="psb", bufs=4, space="PSUM") as psb,
        ):
            scratch = spool.tile([128, TC], bf16)
            xtile = xpool.tile([128, KB, TC], bf16)

            # PE warmup: depends only on the memset, so it runs as soon as
            # the engine prologue ends and un-throttles the HAM clock gate
            # while the startup DMAs are still in flight.
            nc.vector.memset(scratch, 0.0)
            wps = psg.tile([128, TC], f32, tag="gps", name="warmps")
            for _ in range(NWARM):
                nc.tensor.matmul(
                    wps,
                    scratch[:, 0:128],
                    scratch,
                    start=True,
                    stop=True,
                    skip_group_check=True,
                )

            fused = []
            for fb in range(FB):
                w13t = wpool.tile([128, 2, H], bf16, tag="w13t")
                if fb == 0:
                    # Startup split across both HWDGE rings in consumption
                    # order. The scalar ring throttles after ~3 outstanding
                    # DMAs (slot ring waits on completion receipts), so it
                    # gets exactly x q0/q1 + the fb1 weight tile; the sync
                    # ring streams the rest back-to-back.
                    kq, hq = KB // 4, H // 4
                    nc.scalar.dma_start(
                        out=xtile[:, 0:kq, :], in_=xt_d[:, 0:kq, :]
                    )
                    nc.scalar.dma_start(
                        out=xtile[:, kq : 2 * kq, :],
                        in_=xt_d[:, kq : 2 * kq, :],
                    )
                    for q in (0, 1):
                        nc.sync.dma_start(
                            out=w13t[:, :, q * hq : (q + 1) * hq],
                            in_=w13_d[0][:, :, q * hq : (q + 1) * hq],
                        )
                    nc.sync.dma_start(
                        out=xtile[:, 2 * kq : 3 * kq, :],
                        in_=xt_d[:, 2 * kq : 3 * kq, :],
                    )
                    nc.sync.dma_start(
                        out=w13t[:, :, 2 * hq : 3 * hq],
                        in_=w13_d[0][:, :, 2 * hq : 3 * hq],
                    )
                    nc.sync.dma_start(
                        out=xtile[:, 3 * kq :, :], in_=xt_d[:, 3 * kq :, :]
                    )
                    nc.sync.dma_start(
                        out=w13t[:, :, 3 * hq :], in_=w13_d[0][:, :, 3 * hq :]
                    )
                elif fb == 1:
                    nc.scalar.dma_start(out=w13t, in_=w13_d[fb])
                else:
                    nc.sync.dma_start(out=w13t, in_=w13_d[fb])

                # g/u interleaved per k-block: consumption tracks the
                # arrival order of the startup quarters.
                gps = psg.tile([128, TC], f32, tag="gps")
                ups = psu.tile([128, TC], f32, tag="ups")
                for kb in range(KB):
                    nc.tensor.matmul(
                        gps,
                        w13t[:, 0, kb * 128 : (kb + 1) * 128],
                        xtile[:, kb, :],
                        start=(kb == 0),
                        stop=(kb == KB - 1),
                        skip_group_check=True,
                    )
                    nc.tensor.matmul(
                        ups,
                        w13t[:, 1, kb * 128 : (kb + 1) * 128],
                        xtile[:, kb, :],
                        start=(kb == 0),
                        stop=(kb == KB - 1),
                        skip_group_check=True,
                    )

                sg = sgpool.tile([128, TC], f32, tag="sg")
                nc.scalar.activation(
                    sg, gps, mybir.ActivationFunctionType.Silu
                )
                fut = fpool.tile(
                    [128, TC], bf16, tag="fused", name=f"fused{fb}"
                )
                # DVE multiplies u straight out of PSUM; no ACT copy needed.
                nc.vector.tensor_tensor(
                    fut, sg, ups, mybir.AluOpType.mult
                )
                fused.append(fut)

            for hc in range(HCOLS):
                pss = []
                for tb in range(TB):
                    ps = psb.tile(
                        [128, 512], f32, tag="pss", name=f"pss{hc}_{tb}"
                    )
                    pss.append(ps)
                for j in range(W2J):
                    w2t = w2pool.tile([128, W2G, 512], bf16, tag="w2t")
                    nc.sync.dma_start(out=w2t, in_=w2_d[hc, j])
                    for i in range(W2G):
                        fb = W2G * j + i
                        for tb in range(TB):
                            nc.tensor.matmul(
                                pss[tb],
                                fused[fb][:, tb * 128 : (tb + 1) * 128],
                                w2t[:, i, :],
                                start=(fb == 0),
                                stop=(fb == FB - 1),
                            )
                # Evacuations split DVE/ACT so they drain in parallel;
                # stores split across both HWDGE rings to shorten the tail.
                for tb in range(TB):
                    ot = opool.tile(
                        [128, 512], f32, tag="ot", name=f"ot{hc}_{tb}"
                    )
                    if tb % 2 == 0:
                        nc.vector.tensor_copy(ot, pss[tb])
                    else:
                        nc.scalar.activation(
                            ot, pss[tb], mybir.ActivationFunctionType.Copy
                        )
                    eng = nc.sync if tb % 2 == 0 else nc.scalar
                    eng.dma_start(
                        out=out_d[
                            tb * 128 : (tb + 1) * 128,
                            hc * 512 : (hc + 1) * 512,
                        ],
                        in_=ot,
                    )
    nc.finalize()
    return nc


def _dequant(wq, s):
    wq = np.asarray(wq, dtype=np.float32)
    s = np.asarray(s, dtype=np.float32)
    n, k = wq.shape
    nb, kb = s.shape
    w = wq.reshape(nb, n // nb, kb, k // kb) * s[:, None, :, None]
    return w.reshape(n, k)


def _prep_inputs(hidden_states, w1_q, w1_s, w3_q, w3_s, w2_q, w2_s):
    import ml_dtypes

    bf = ml_dtypes.bfloat16

    w1 = _dequant(w1_q, w1_s).astype(bf)  # [F, H]
    w3 = _dequant(w3_q, w3_s).astype(bf)  # [F, H]
    w2 = _dequant(w2_q, w2_s).astype(bf)  # [H, F]

    # w1p[fb, p, kb*128+c] = w1[fb*128+c, kb*128+p]  (and same for w3);
    # interleaved per partition: w13p[fb, p, 0] = w1 row, [fb, p, 1] = w3.
    w1p = w1.reshape(FB, 128, KB, 128).transpose(0, 3, 2, 1).reshape(FB, 128, H)
    w3p = w3.reshape(FB, 128, KB, 128).transpose(0, 3, 2, 1).reshape(FB, 128, H)
    w13p = np.ascontiguousarray(np.stack([w1p, w3p], axis=2))  # [FB,128,2,H]

    # w2p[hc, j, p, i, c] = w2[hc*512+c, (W2G*j+i)*128+p]
    w2p = np.ascontiguousarray(
        np.asarray(w2).reshape(HCOLS, 512, W2J, W2G, 128).transpose(0, 2, 4, 3, 1)
    )

    x = np.asarray(hidden_states, dtype=np.float32).astype(bf)
    xts = []
    for c in range(NCORES):
        xc = x[c * TC : (c + 1) * TC, :]
        # xt[p, kb, t] = xc[t, kb*128+p] - partition-major, so the whole
        # 2MB x-transpose lands in one DMA with 16KB/partition contiguous.
        xts.append(
            np.ascontiguousarray(xc.reshape(TC, KB, 128).transpose(2, 1, 0))
        )

    return [
        {"xt": xts[c], "w13p": w13p, "w2p": w2p}
        for c in range(NCORES)
    ]


def _run(in_maps, **kwargs):
    from concourse.bass_utils import run_bass_kernel_spmd

    if "nc" not in _CACHE:
        _CACHE["nc"] = _build_program()
    res = run_bass_kernel_spmd(
        _CACHE["nc"], in_maps, list(range(NCORES)), **kwargs
    )
    out = np.concatenate(
        [res.results[c]["out"] for c in range(NCORES)], axis=0
    )
    return np.asarray(out, dtype=np.float32), res


def kernel(hidden_states, w1_q, w1_s, w3_q, w3_s, w2_q, w2_s):
    in_maps = _prep_inputs(
        hidden_states, w1_q, w1_s, w3_q, w3_s, w2_q, w2_s
    )
    out, _ = _run(in_maps)
    return out
